# revision 24
# baseline (speedup 1.0000x reference)
"""Self-contained Trainium2 Bass kernel for batched multi-head attention
with interleaved RoPE and a block-causal mask (block size 8).

Shapes (hardcoded): x [8, 1024, 1024] f32, weights [1024, 1024] f32,
freqs_cos/sin [1024, 32] f32 -> out [8, 1024, 1024] f32.

Sharding: data-parallel over batch, one batch element per NeuronCore (8 cores).

Device algorithm (per core, matmuls in bf16):
  - host pre-transposes x -> XT [D, S] and de-interleaves the RoPE pairing by
    permuting wq/wk columns so each head's 64 dims are [32 real | 32 imag].
  - warmup: a short chain of dummy matmuls keeps the PE HAM clock-gate busy
    while the first input DMAs land, so real matmuls start at 2.4 GHz.
  - QT = Wq^T XT, KT = Wk^T XT  ([D, S] layouts, head-major rows)
  - RoPE in [d, s] layout: rot = t * cosf + swap32(t) * sinf, with the 32-row
    block swap done by SBUF->SBUF DMA and sign folded into the sinf table.
  - V = XT^T Wv in natural [S, D] layout, stored with a ones-column per head
    (V' [S, 65] per head) so the PV matmul also produces the softmax
    denominator as its row 64.
  - attention per head-PAIR (the two heads of a 128-row partition tile) per
    512-wide q-bank jb: for each k-tile i, both heads' transposed score
    tiles ST[k, q] go side by side into ONE PSUM pair-tile [128, 2w]
    (h0 rows of kt use PE row-groups 0-1, h1 rows 2-3), then ONE exp
    (ACT, scale 1/8 folded) covers both heads, ONE tensor_tensor applies
    the block-diagonal mask to both heads via a 3-D access pattern, and
    two PV matmuls accumulate otT[h] = V'^T @ PT per head ([65, 512]).
    The ST matmul for k-tile i+1 is issued before PV of k-tile i so the
    PE streams scores while ACT runs exp (2-deep software pipeline).
  - normalization per head-bank: rec = reciprocal_approx_fast(den row read
    straight from PSUM), partition-broadcast on GPSIMD, fused TT multiply
    PSUM->SBUF.
  - final = outT^T @ Wo streamed back to HBM in f32 (PSUM->SBUF copies on
    the scalar engine to keep DVE free).

PSUM budget (8 banks): 2 projection accumulators + 4 for double-buffered
score pair-tiles + 2 for the per-head otT accumulators.
"""

import os
import sys
import types

import numpy as np

B, S, D, H, HD, BS = 8, 1024, 1024, 16, 64, 8
P = 128
NT = D // P  # 8 partition tiles
NCORES = 8

LAST_RESULT = None  # BassKernelResults of the most recent run (for test harness)


def _install_axon_hooks():
    """Provide antenv.axon_hooks (NTFF profiling hook) when the image lacks it."""
    if "antenv.axon_hooks" in sys.modules:
        return
    try:
        import antenv
        from trn_agent_boot.trn_boot import _ntff_profile_via_ctypes

        mod = types.ModuleType("antenv.axon_hooks")
        hook = _ntff_profile_via_ctypes("/opt/axon/libaxon_pjrt.so")
        mod.get_axon_ntff_profile_hook = lambda: hook
        mod.set_axon_ntff_profile_hook = lambda h: None
        sys.modules["antenv.axon_hooks"] = mod
        antenv.axon_hooks = mod
    except Exception:
        mod = types.ModuleType("antenv.axon_hooks")
        mod.get_axon_ntff_profile_hook = lambda: None
        mod.set_axon_ntff_profile_hook = lambda h: None
        sys.modules["antenv.axon_hooks"] = mod


_NC_CACHE = {}


def _build_nc():
    """Build and compile the Bass graph (one SPMD program for all 8 cores)."""
    if "nc" in _NC_CACHE:
        return _NC_CACHE["nc"]

    import concourse.mybir as mybir
    import concourse.tile as tile
    from concourse import bacc

    BF = mybir.dt.bfloat16
    F32 = mybir.dt.float32
    MUL = mybir.AluOpType.mult
    ADD = mybir.AluOpType.add
    EXP = mybir.ActivationFunctionType.Exp
    COPY = mybir.ActivationFunctionType.Copy

    nc = bacc.Bacc("TRN2", target_bir_lowering=False, debug=False)

    xt_d = nc.dram_tensor("xt", [D, S], BF, kind="ExternalInput")
    wq_d = nc.dram_tensor("wq", [D, D], BF, kind="ExternalInput")
    wk_d = nc.dram_tensor("wk", [D, D], BF, kind="ExternalInput")
    wv_d = nc.dram_tensor("wv", [D, D], BF, kind="ExternalInput")
    wo_d = nc.dram_tensor("wo", [D, D], BF, kind="ExternalInput")
    cos_d = nc.dram_tensor("cosf", [P, S], BF, kind="ExternalInput")
    sin_d = nc.dram_tensor("sinf", [P, S], BF, kind="ExternalInput")
    mask_d = nc.dram_tensor("mask", [P, 2 * P], BF, kind="ExternalInput")
    out_d = nc.dram_tensor("out", [S, D], F32, kind="ExternalOutput")

    HC = HD + 1  # 65: V columns per head incl. the ones column

    with tile.TileContext(nc) as tc:
        with (
            tc.tile_pool(name="big", bufs=1) as big,
            tc.tile_pool(name="prj", bufs=2, space="PSUM") as prj,
            tc.tile_pool(name="stp", bufs=2, space="PSUM") as stp,
            tc.tile_pool(name="otp", bufs=2, space="PSUM") as otp,
            tc.tile_pool(name="work", bufs=2) as work,
            tc.tile_pool(name="ptp", bufs=3) as ptp,
        ):
            xt = [big.tile([P, S], BF, tag=f"xt{j}", name=f"xt{j}") for j in range(NT)]
            wqt = [big.tile([P, D], BF, tag=f"wq{j}", name=f"wq{j}") for j in range(NT)]
            wkt = [big.tile([P, D], BF, tag=f"wk{j}", name=f"wk{j}") for j in range(NT)]
            wvt = [big.tile([P, D], BF, tag=f"wv{j}", name=f"wv{j}") for j in range(NT)]
            wot = [big.tile([P, D], BF, tag=f"wo{j}", name=f"wo{j}") for j in range(NT)]
            qt = [big.tile([P, S], BF, tag=f"qt{t}", name=f"qt{t}") for t in range(NT)]
            kt = [big.tile([P, S], BF, tag=f"kt{t}", name=f"kt{t}") for t in range(NT)]
            vs = [big.tile([P, H * HC], BF, tag=f"vs{t}", name=f"vs{t}") for t in range(NT)]
            ot = [big.tile([P, S], BF, tag=f"ot{t}", name=f"ot{t}") for t in range(NT)]
            cosf = big.tile([P, S], BF, tag="cosf", name="cosf")
            sinf = big.tile([P, S], BF, tag="sinf", name="sinf")
            maskt2 = big.tile([P, 2 * P], BF, tag="mask", name="mask")
            wup = big.tile([P, 512], BF, tag="wup", name="wup")

            # ---- PE warmup: dummy matmul chain with no input deps ----------
            nc.vector.memset(wup[:], 0.0)
            wps = prj.tile([P, 512], F32, tag="prj", name="wps")
            for k in range(22):
                nc.tensor.matmul(
                    wps[:], wup[:, 0:P], wup[:, 0:512],
                    start=(k == 0), stop=(k == 21),
                )
            wsb = work.tile([1, P], F32, tag="wsb", name="wsb")
            nc.vector.tensor_copy(wsb[:], wps[0:1, 0:P])

            # ---- input DMAs in consumption order --------------------------
            for j in range(NT):
                rs = slice(j * P, (j + 1) * P)
                nc.sync.dma_start(xt[j][:], xt_d[rs, :])
                nc.sync.dma_start(wvt[j][:], wv_d[rs, :])
            for j in range(NT):
                rs = slice(j * P, (j + 1) * P)
                nc.sync.dma_start(wqt[j][:], wq_d[rs, :])
                nc.sync.dma_start(wkt[j][:], wk_d[rs, :])
            nc.sync.dma_start(cosf[:], cos_d[:])
            nc.sync.dma_start(sinf[:], sin_d[:])
            nc.sync.dma_start(maskt2[:], mask_d[:])
            for j in range(NT):
                rs = slice(j * P, (j + 1) * P)
                nc.sync.dma_start(wot[j][:], wo_d[rs, :])

            for t in range(NT):
                nc.vector.memset(
                    vs[t].rearrange("p (h c) -> p h c", c=HC)[:, :, HD : HD + 1], 1.0
                )

            # ---- V projection --------------------------------------------
            def v_group(t, m):
                cs = slice(t * P, (t + 1) * P)
                sl = slice(m * 512, (m + 1) * 512)
                pv = prj.tile([P, 512], F32, tag="prj", name="pv")
                for j in range(NT):
                    nc.tensor.matmul(
                        pv[:], xt[j][:, cs], wvt[j][:, sl],
                        start=(j == 0), stop=(j == NT - 1),
                    )
                dst = vs[t].rearrange("p (h c) -> p h c", c=HC)[
                    :, m * 8 : (m + 1) * 8, 0:HD
                ]
                srcv = pv.rearrange("p (h c) -> p h c", c=HD)
                nc.vector.tensor_copy(dst, srcv)

            # first four groups (t=0,1 x m=0,1) j-interleaved so matmuls
            # trickle in densely as the per-j DMAs land (keeps HAM warm);
            # two accumulators borrow idle score-pool banks
            pvs = [
                prj.tile([P, 512], F32, tag="prj", name="pv"),
                prj.tile([P, 512], F32, tag="prj", name="pv"),
                stp.tile([P, 512], F32, tag="st2", name="pv"),
                stp.tile([P, 512], F32, tag="st2", name="pv"),
            ]
            for j in range(NT):
                for g, (t, m) in enumerate(((0, 0), (0, 1), (1, 0), (1, 1))):
                    nc.tensor.matmul(
                        pvs[g][:], xt[j][:, t * P : (t + 1) * P],
                        wvt[j][:, m * 512 : (m + 1) * 512],
                        start=(j == 0), stop=(j == NT - 1),
                    )
            for g, (t, m) in enumerate(((0, 0), (0, 1), (1, 0), (1, 1))):
                dst = vs[t].rearrange("p (h c) -> p h c", c=HC)[
                    :, m * 8 : (m + 1) * 8, 0:HD
                ]
                nc.vector.tensor_copy(dst, pvs[g].rearrange("p (h c) -> p h c", c=HD))
            for t in range(2, NT):
                for m in range(2):
                    v_group(t, m)

            # RoPE helper: per 128-row tile the layout is [h0r, h0i, h1r,
            # h1i] (32 rows each); rot = t*cosf + swap32(t)*sinf (sinf
            # carries the sign)
            def rope(buf_t):
                tr = work.tile([P, S], BF, tag="trot", name="trot")
                for b4 in range(4):
                    sblk = (b4 ^ 1) * 32
                    dblk = b4 * 32
                    nc.sync.dma_start(
                        tr[dblk : dblk + 32, :], buf_t[sblk : sblk + 32, :]
                    )
                nc.vector.tensor_tensor(tr[:], tr[:], sinf[:], op=MUL)
                nc.vector.tensor_tensor(buf_t[:], buf_t[:], cosf[:], op=MUL)
                nc.vector.tensor_tensor(buf_t[:], buf_t[:], tr[:], op=ADD)

            def qk_feed(t, which):
                """Thunks for one Q-or-K projection of tile t: 2 m-groups of
                8 chained matmuls + PSUM->SBUF cast, then RoPE. Drained one
                thunk at a time inside the attention k-loop so the in-order
                PE queue always has independent matmuls before each blocking
                PV matmul."""
                cs = slice(t * P, (t + 1) * P)
                wsrc = wqt if which == "q" else wkt
                dstt = qt[t] if which == "q" else kt[t]
                thunks = []
                cell = {}
                for m in range(2):
                    sl = slice(m * 512, (m + 1) * 512)

                    def mk_mm(j, m=m, sl=sl):
                        def f():
                            if j == 0:
                                cell[m] = prj.tile(
                                    [P, 512], F32, tag="prj", name="pq"
                                )
                            nc.tensor.matmul(
                                cell[m][:], wsrc[j][:, cs], xt[j][:, sl],
                                start=(j == 0), stop=(j == NT - 1),
                            )
                        return f

                    for j in range(NT):
                        thunks.append(mk_mm(j))

                    def mk_cast(m=m, sl=sl):
                        def f():
                            nc.vector.tensor_copy(dstt[:, sl], cell[m][:])
                        return f

                    thunks.append(mk_cast())
                thunks.append(lambda: rope(dstt))
                return thunks

            # ---- output projection pieces (fed partially into the last
            # attention pair): final[s, :] = sum_i ot[i][:, s]^T wo[i]
            oproj_state = {}

            def oproj_mm(st, m, i):
                key = (st, m)
                if key not in oproj_state:
                    oproj_state[key] = prj.tile([P, 512], F32, tag="prj", name="fp")
                fp = oproj_state[key]
                nc.tensor.matmul(
                    fp[:],
                    ot[i][:, st * P : (st + 1) * P],
                    wot[i][:, m * 512 : (m + 1) * 512],
                    start=(i == 0), stop=(i == NT - 1),
                )
                if i == NT - 1:
                    osb = work.tile([P, 512], F32, tag="osb", name="osb")
                    nc.scalar.activation(osb[:], fp[:], COPY)
                    nc.sync.dma_start(
                        out_d[st * P : (st + 1) * P, m * 512 : (m + 1) * 512],
                        osb[:],
                    )

            # ---- attention per head-pair, per 512-wide q-bank jb ----------
            scale = 1.0 / 8.0
            maskv = maskt2.rearrange("p (h c) -> p h c", c=P)

            def attn_half(t, jb, feed, pace):
                ilast = 4 * jb + 3 if jb == 0 else NT - 1
                ks = list(range(ilast + 1))
                widths = [512 - max(0, 128 * i - 512 * jb) for i in ks]
                qoffs = [max(512 * jb, 128 * i) for i in ks]
                otph = [
                    otp.tile([HC, 512], F32, tag="ot", name=f"otp{h}")
                    for h in range(2)
                ]
                st2s = {}
                HO = 512  # h1's half starts at a PSUM bank boundary

                def issue_st(i):
                    w = widths[i]
                    st2 = stp.tile([P, 2 * HO], F32, tag="st2", name="st2")
                    st2s[i] = st2
                    for h in range(2):
                        base = h * HD
                        nc.tensor.matmul(
                            st2[:, h * HO : h * HO + w],
                            kt[t][base : base + HD, 128 * i : 128 * (i + 1)],
                            qt[t][base : base + HD, qoffs[i] : qoffs[i] + w],
                            start=True, stop=True,
                        )

                issue_st(0)
                for i in ks:
                    w = widths[i]
                    st2 = st2s.pop(i)
                    pt2 = ptp.tile([P, 2 * HO], BF, tag="pt2", name="pt2")
                    nc.scalar.activation(
                        pt2.rearrange("p (h c) -> p h c", c=HO)[:, :, 0:w],
                        st2.rearrange("p (h c) -> p h c", c=HO)[:, :, 0:w],
                        EXP, scale=scale,
                    )
                    if 128 * i >= 512 * jb:
                        # diagonal block sits at local cols [0:128] of both
                        # head-halves: one 3-D-AP tensor_tensor
                        pv2 = pt2.rearrange("p (h c) -> p h c", c=HO)[:, :, 0:P]
                        nc.vector.tensor_tensor(pv2, pv2, maskv, op=MUL)
                    if i < ilast:
                        issue_st(i + 1)
                    for _ in range(pace):
                        if feed:
                            feed.pop(0)()
                    for h in range(2):
                        hh = 2 * t + h
                        o = qoffs[i] - 512 * jb
                        nc.tensor.matmul(
                            otph[h][:, o : o + w],
                            vs[i][:, hh * HC : (hh + 1) * HC],
                            pt2[:, h * HO : h * HO + w],
                            start=(i == 0), stop=(i == ilast),
                        )
                while feed:
                    feed.pop(0)()
                for h in range(2):
                    base = h * HD
                    den = work.tile([1, 512], F32, tag="den", name="den")
                    nc.scalar.copy(den[:], otph[h][HD : HD + 1, :])
                    rec = work.tile([1, 512], F32, tag="rec", name="rec")
                    nc.vector.reciprocal_approx_fast(rec[:], den[:])
                    bc = work.tile([HD, 512], F32, tag="bc", name="bc")
                    nc.gpsimd.partition_broadcast(bc[:], rec[:])
                    nc.vector.tensor_tensor(
                        ot[t][base : base + HD, 512 * jb : 512 * (jb + 1)],
                        otph[h][0:HD, :], bc[:],
                        op=MUL,
                    )

            # steady-state pipeline: Q/K projections of tile t+1 (and, for
            # the last tile, the first output-projection chains) are drained
            # into attention of tile t between the score and PV matmuls
            for th in qk_feed(0, "q"):
                th()
            for th in qk_feed(0, "k"):
                th()
            for t in range(NT):
                if t + 1 < NT:
                    fq, fk = qk_feed(t + 1, "q"), qk_feed(t + 1, "k")
                else:
                    fq = [
                        (lambda st=st, m=m, i=i: oproj_mm(st, m, i))
                        for (st, m) in ((0, 0), (0, 1))
                        for i in range(NT - 1)
                    ]
                    fk = []
                attn_half(t, 0, fq, 5)
                attn_half(t, 1, fk, 3)

            for (st, m) in ((0, 0), (0, 1)):
                oproj_mm(st, m, NT - 1)
            for st in range(NT):
                for m in range(2):
                    if (st, m) in ((0, 0), (0, 1)):
                        continue
                    for i in range(NT):
                        oproj_mm(st, m, i)

    nc.compile()
    _NC_CACHE["nc"] = nc
    return nc


def _host_prep(x, wq, wk, wv, wo, freqs_cos, freqs_sin):
    import ml_dtypes

    bf16 = ml_dtypes.bfloat16

    # de-interleave RoPE pairs: permuted col c of head h maps to original
    # column h*64 + (2r if r<32 else 2(r-32)+1)
    r = np.arange(HD)
    src_local = np.where(r < 32, 2 * r, 2 * (r - 32) + 1)
    perm = (np.arange(H)[:, None] * HD + src_local[None, :]).reshape(-1)

    wq_p = np.ascontiguousarray(wq[:, perm]).astype(bf16)
    wk_p = np.ascontiguousarray(wk[:, perm]).astype(bf16)
    wv_c = np.ascontiguousarray(wv).astype(bf16)
    wo_c = np.ascontiguousarray(wo).astype(bf16)

    cos_t = np.ascontiguousarray(freqs_cos.T).astype(np.float32)  # [32, S]
    sin_t = np.ascontiguousarray(freqs_sin.T).astype(np.float32)
    cosf = np.concatenate([cos_t, cos_t, cos_t, cos_t], 0).astype(bf16)  # [128,S]
    sinf = np.concatenate([-sin_t, sin_t, -sin_t, sin_t], 0).astype(bf16)

    kq = np.arange(P)
    mask = ((kq[:, None] // BS) <= (kq[None, :] // BS)).astype(bf16)  # [128,128]
    mask2 = np.concatenate([mask, mask], axis=1)  # [128, 256]

    in_maps = []
    for b in range(NCORES):
        xt = np.ascontiguousarray(x[b].T).astype(bf16)  # [D, S]
        in_maps.append(
            {
                "xt": xt,
                "wq": wq_p,
                "wk": wk_p,
                "wv": wv_c,
                "wo": wo_c,
                "cosf": cosf,
                "sinf": sinf,
                "mask": mask2,
            }
        )
    return in_maps


def kernel(x, wq, wk, wv, wo, freqs_cos, freqs_sin):
    global LAST_RESULT
    x = np.asarray(x, dtype=np.float32)
    wq = np.asarray(wq, dtype=np.float32)
    wk = np.asarray(wk, dtype=np.float32)
    wv = np.asarray(wv, dtype=np.float32)
    wo = np.asarray(wo, dtype=np.float32)
    freqs_cos = np.asarray(freqs_cos, dtype=np.float32)
    freqs_sin = np.asarray(freqs_sin, dtype=np.float32)

    trace = bool(os.environ.get("BASS_TRACE"))
    if trace:
        _install_axon_hooks()
        import concourse.bass_utils as bass_utils

        bass_utils.upload_artifacts = lambda tmpdir: tmpdir  # no-egress sandbox

    from concourse.bass_utils import run_bass_kernel_spmd

    nc = _build_nc()
    in_maps = _host_prep(x, wq, wk, wv, wo, freqs_cos, freqs_sin)
    res = run_bass_kernel_spmd(
        nc, in_maps, core_ids=list(range(NCORES)), trace=trace
    )
    LAST_RESULT = res
    out = np.stack([res.results[b]["out"] for b in range(NCORES)], 0)
    return out.astype(np.float32)


# revision 25
# speedup vs baseline: 1.1777x; 1.1777x over previous
"""Self-contained Trainium2 Bass kernel for batched multi-head attention
with interleaved RoPE and a block-causal mask (block size 8).

Shapes (hardcoded): x [8, 1024, 1024] f32, weights [1024, 1024] f32,
freqs_cos/sin [1024, 32] f32 -> out [8, 1024, 1024] f32.

Sharding: data-parallel over batch, one batch element per NeuronCore (8 cores).

Device algorithm (per core, matmuls in bf16):
  - host pre-transposes x -> XT [D, S] and de-interleaves the RoPE pairing by
    permuting wq/wk columns so each head's 64 dims are [32 real | 32 imag].
  - warmup: a short chain of dummy matmuls keeps the PE HAM clock-gate busy
    while the first input DMAs land, so real matmuls start at 2.4 GHz.
  - QT = Wq^T XT, KT = Wk^T XT  ([D, S] layouts, head-major rows)
  - RoPE in [d, s] layout: rot = t * cosf + swap32(t) * sinf, with the 32-row
    block swap done by SBUF->SBUF DMA and sign folded into the sinf table.
  - V = XT^T Wv in natural [S, D] layout, stored with a ones-column per head
    (V' [S, 65] per head) so the PV matmul also produces the softmax
    denominator as its row 64.
  - attention per head-PAIR (the two heads of a 128-row partition tile) per
    512-wide q-bank jb: for each k-tile i, both heads' transposed score
    tiles ST[k, q] go side by side into ONE PSUM pair-tile [128, 2w]
    (h0 rows of kt use PE row-groups 0-1, h1 rows 2-3), then ONE exp
    (ACT, scale 1/8 folded) covers both heads, ONE tensor_tensor applies
    the block-diagonal mask to both heads via a 3-D access pattern, and
    two PV matmuls accumulate otT[h] = V'^T @ PT per head ([65, 512]).
    The ST matmul for k-tile i+1 is issued before PV of k-tile i so the
    PE streams scores while ACT runs exp (2-deep software pipeline).
  - normalization per head-bank: rec = reciprocal_approx_fast(den row read
    straight from PSUM), partition-broadcast on GPSIMD, fused TT multiply
    PSUM->SBUF.
  - final = outT^T @ Wo streamed back to HBM in f32 (PSUM->SBUF copies on
    the scalar engine to keep DVE free).

PSUM budget (8 banks): 2 projection accumulators + 4 for double-buffered
score pair-tiles + 2 for the per-head otT accumulators.
"""

import os
import sys
import types

import numpy as np

B, S, D, H, HD, BS = 8, 1024, 1024, 16, 64, 8
P = 128
NT = D // P  # 8 partition tiles
NCORES = 8

LAST_RESULT = None  # BassKernelResults of the most recent run (for test harness)


def _install_axon_hooks():
    """Provide antenv.axon_hooks (NTFF profiling hook) when the image lacks it."""
    if "antenv.axon_hooks" in sys.modules:
        return
    try:
        import antenv
        from trn_agent_boot.trn_boot import _ntff_profile_via_ctypes

        mod = types.ModuleType("antenv.axon_hooks")
        hook = _ntff_profile_via_ctypes("/opt/axon/libaxon_pjrt.so")
        mod.get_axon_ntff_profile_hook = lambda: hook
        mod.set_axon_ntff_profile_hook = lambda h: None
        sys.modules["antenv.axon_hooks"] = mod
        antenv.axon_hooks = mod
    except Exception:
        mod = types.ModuleType("antenv.axon_hooks")
        mod.get_axon_ntff_profile_hook = lambda: None
        mod.set_axon_ntff_profile_hook = lambda h: None
        sys.modules["antenv.axon_hooks"] = mod


_NC_CACHE = {}


def _build_nc():
    """Build and compile the Bass graph (one SPMD program for all 8 cores)."""
    if "nc" in _NC_CACHE:
        return _NC_CACHE["nc"]

    import concourse.mybir as mybir
    import concourse.tile as tile
    from concourse import bacc

    BF = mybir.dt.bfloat16
    F32 = mybir.dt.float32
    MUL = mybir.AluOpType.mult
    ADD = mybir.AluOpType.add
    EXP = mybir.ActivationFunctionType.Exp
    COPY = mybir.ActivationFunctionType.Copy

    nc = bacc.Bacc("TRN2", target_bir_lowering=False, debug=False)

    xt_d = nc.dram_tensor("xt", [D, S], BF, kind="ExternalInput")
    wq_d = nc.dram_tensor("wq", [D, D], BF, kind="ExternalInput")
    wk_d = nc.dram_tensor("wk", [D, D], BF, kind="ExternalInput")
    wv_d = nc.dram_tensor("wv", [D, D], BF, kind="ExternalInput")
    wo_d = nc.dram_tensor("wo", [D, D], BF, kind="ExternalInput")
    cos_d = nc.dram_tensor("cosf", [P, S], BF, kind="ExternalInput")
    sin_d = nc.dram_tensor("sinf", [P, S], BF, kind="ExternalInput")
    mask_d = nc.dram_tensor("mask", [P, 2 * P], BF, kind="ExternalInput")
    out_d = nc.dram_tensor("out", [S, D], F32, kind="ExternalOutput")

    HC = HD + 1  # 65: V columns per head incl. the ones column

    with tile.TileContext(nc) as tc:
        with (
            tc.tile_pool(name="big", bufs=1) as big,
            tc.tile_pool(name="prj", bufs=2, space="PSUM") as prj,
            tc.tile_pool(name="stp", bufs=2, space="PSUM") as stp,
            tc.tile_pool(name="otp", bufs=2, space="PSUM") as otp,
            tc.tile_pool(name="work", bufs=2) as work,
            tc.tile_pool(name="ptp", bufs=3) as ptp,
        ):
            xt = [big.tile([P, S], BF, tag=f"xt{j}", name=f"xt{j}") for j in range(NT)]
            wqt = [big.tile([P, D], BF, tag=f"wq{j}", name=f"wq{j}") for j in range(NT)]
            wkt = [big.tile([P, D], BF, tag=f"wk{j}", name=f"wk{j}") for j in range(NT)]
            wvt = [big.tile([P, D], BF, tag=f"wv{j}", name=f"wv{j}") for j in range(NT)]
            wot = [big.tile([P, D], BF, tag=f"wo{j}", name=f"wo{j}") for j in range(NT)]
            qt = [big.tile([P, S], BF, tag=f"qt{t}", name=f"qt{t}") for t in range(NT)]
            kt = [big.tile([P, S], BF, tag=f"kt{t}", name=f"kt{t}") for t in range(NT)]
            vs = [big.tile([P, H * HC], BF, tag=f"vs{t}", name=f"vs{t}") for t in range(NT)]
            ot = [big.tile([P, S], BF, tag=f"ot{t}", name=f"ot{t}") for t in range(NT)]
            cosf = big.tile([P, S], BF, tag="cosf", name="cosf")
            sinf = big.tile([P, S], BF, tag="sinf", name="sinf")
            maskt2 = big.tile([P, 2 * P], BF, tag="mask", name="mask")
            wup = big.tile([P, 512], BF, tag="wup", name="wup")

            # ---- PE warmup: dummy matmul chain with no input deps ----------
            nc.vector.memset(wup[:], 0.0)
            wps = prj.tile([P, 512], F32, tag="prj", name="wps")
            for k in range(22):
                nc.tensor.matmul(
                    wps[:], wup[:, 0:P], wup[:, 0:512],
                    start=(k == 0), stop=(k == 21),
                )
            wsb = work.tile([1, P], F32, tag="wsb", name="wsb")
            nc.vector.tensor_copy(wsb[:], wps[0:1, 0:P])

            # ---- input DMAs in consumption order --------------------------
            for j in range(NT):
                rs = slice(j * P, (j + 1) * P)
                nc.sync.dma_start(xt[j][:], xt_d[rs, :])
                nc.sync.dma_start(wvt[j][:], wv_d[rs, :])
            for j in range(NT):
                rs = slice(j * P, (j + 1) * P)
                nc.sync.dma_start(wqt[j][:], wq_d[rs, :])
                nc.sync.dma_start(wkt[j][:], wk_d[rs, :])
            nc.sync.dma_start(cosf[:], cos_d[:])
            nc.sync.dma_start(sinf[:], sin_d[:])
            nc.sync.dma_start(maskt2[:], mask_d[:])
            for j in range(NT):
                rs = slice(j * P, (j + 1) * P)
                nc.sync.dma_start(wot[j][:], wo_d[rs, :])

            for t in range(NT):
                nc.vector.memset(
                    vs[t].rearrange("p (h c) -> p h c", c=HC)[:, :, HD : HD + 1], 1.0
                )

            # ---- V projection --------------------------------------------
            def v_group(t, m):
                cs = slice(t * P, (t + 1) * P)
                sl = slice(m * 512, (m + 1) * 512)
                pv = prj.tile([P, 512], F32, tag="prj", name="pv")
                for j in range(NT):
                    nc.tensor.matmul(
                        pv[:], xt[j][:, cs], wvt[j][:, sl],
                        start=(j == 0), stop=(j == NT - 1),
                    )
                dst = vs[t].rearrange("p (h c) -> p h c", c=HC)[
                    :, m * 8 : (m + 1) * 8, 0:HD
                ]
                srcv = pv.rearrange("p (h c) -> p h c", c=HD)
                nc.vector.tensor_copy(dst, srcv)

            # first four groups (t=0,1 x m=0,1) j-interleaved so matmuls
            # trickle in densely as the per-j DMAs land (keeps HAM warm);
            # two accumulators borrow idle score-pool banks
            pvs = [
                prj.tile([P, 512], F32, tag="prj", name="pv"),
                prj.tile([P, 512], F32, tag="prj", name="pv"),
                stp.tile([P, 512], F32, tag="st2", name="pv"),
                stp.tile([P, 512], F32, tag="st2", name="pv"),
            ]
            for j in range(NT):
                for g, (t, m) in enumerate(((0, 0), (0, 1), (1, 0), (1, 1))):
                    nc.tensor.matmul(
                        pvs[g][:], xt[j][:, t * P : (t + 1) * P],
                        wvt[j][:, m * 512 : (m + 1) * 512],
                        start=(j == 0), stop=(j == NT - 1),
                    )
            for g, (t, m) in enumerate(((0, 0), (0, 1), (1, 0), (1, 1))):
                dst = vs[t].rearrange("p (h c) -> p h c", c=HC)[
                    :, m * 8 : (m + 1) * 8, 0:HD
                ]
                nc.vector.tensor_copy(dst, pvs[g].rearrange("p (h c) -> p h c", c=HD))
            for t in range(2, NT):
                for m in range(2):
                    v_group(t, m)

            # RoPE helper: per 128-row tile the layout is [h0r, h0i, h1r,
            # h1i] (32 rows each); rot = t*cosf + swap32(t)*sinf (sinf
            # carries the sign)
            def rope(buf_t):
                tr = work.tile([P, S], BF, tag="trot", name="trot")
                for b4 in range(4):
                    sblk = (b4 ^ 1) * 32
                    dblk = b4 * 32
                    nc.sync.dma_start(
                        tr[dblk : dblk + 32, :], buf_t[sblk : sblk + 32, :]
                    )
                nc.vector.tensor_tensor(tr[:], tr[:], sinf[:], op=MUL)
                nc.vector.tensor_tensor(buf_t[:], buf_t[:], cosf[:], op=MUL)
                nc.vector.tensor_tensor(buf_t[:], buf_t[:], tr[:], op=ADD)

            def qk_feed(t, which):
                """Thunks for one Q-or-K projection of tile t: 2 m-groups of
                8 chained matmuls + PSUM->SBUF cast, then RoPE. Drained one
                thunk at a time inside the attention k-loop so the in-order
                PE queue always has independent matmuls before each blocking
                PV matmul."""
                cs = slice(t * P, (t + 1) * P)
                wsrc = wqt if which == "q" else wkt
                dstt = qt[t] if which == "q" else kt[t]
                thunks = []
                cell = {}
                for m in range(2):
                    sl = slice(m * 512, (m + 1) * 512)

                    def mk_mm(j, m=m, sl=sl):
                        def f():
                            if j == 0:
                                cell[m] = prj.tile(
                                    [P, 512], F32, tag="prj", name="pq"
                                )
                            nc.tensor.matmul(
                                cell[m][:], wsrc[j][:, cs], xt[j][:, sl],
                                start=(j == 0), stop=(j == NT - 1),
                            )
                        return f

                    for j in range(NT):
                        thunks.append(mk_mm(j))

                    def mk_cast(m=m, sl=sl):
                        def f():
                            nc.vector.tensor_copy(dstt[:, sl], cell[m][:])
                        return f

                    thunks.append(mk_cast())
                thunks.append(lambda: rope(dstt))
                return thunks

            # ---- output projection pieces (fed partially into the last
            # attention pair): final[s, :] = sum_i ot[i][:, s]^T wo[i]
            oproj_state = {}

            def oproj_mm(st, m, i):
                key = (st, m)
                if key not in oproj_state:
                    oproj_state[key] = prj.tile([P, 512], F32, tag="prj", name="fp")
                fp = oproj_state[key]
                nc.tensor.matmul(
                    fp[:],
                    ot[i][:, st * P : (st + 1) * P],
                    wot[i][:, m * 512 : (m + 1) * 512],
                    start=(i == 0), stop=(i == NT - 1),
                )
                if i == NT - 1:
                    osb = work.tile([P, 512], F32, tag="osb", name="osb")
                    nc.scalar.activation(osb[:], fp[:], COPY)
                    nc.sync.dma_start(
                        out_d[st * P : (st + 1) * P, m * 512 : (m + 1) * 512],
                        osb[:],
                    )

            # ---- attention per head-pair, per 512-wide q-bank jb ----------
            scale = 1.0 / 8.0
            maskv = maskt2.rearrange("p (h c) -> p h c", c=P)

            def attn_half(t, jb, feed, pace):
                ilast = 4 * jb + 3 if jb == 0 else NT - 1
                ks = list(range(ilast + 1))
                widths = [512 - max(0, 128 * i - 512 * jb) for i in ks]
                qoffs = [max(512 * jb, 128 * i) for i in ks]
                otph = [
                    otp.tile([HC, 512], F32, tag="ot", name=f"otp{h}")
                    for h in range(2)
                ]
                st2s = {}
                HO = 512  # h1's half starts at a PSUM bank boundary

                def issue_st(i):
                    w = widths[i]
                    st2 = stp.tile([P, 2 * HO], F32, tag="st2", name="st2")
                    st2s[i] = st2
                    for h in range(2):
                        base = h * HD
                        nc.tensor.matmul(
                            st2[:, h * HO : h * HO + w],
                            kt[t][base : base + HD, 128 * i : 128 * (i + 1)],
                            qt[t][base : base + HD, qoffs[i] : qoffs[i] + w],
                            start=True, stop=True,
                        )

                issue_st(0)
                for i in ks:
                    w = widths[i]
                    st2 = st2s.pop(i)
                    pt2 = ptp.tile([P, 2 * HO], BF, tag="pt2", name="pt2")
                    nc.scalar.activation(
                        pt2.rearrange("p (h c) -> p h c", c=HO)[:, :, 0:w],
                        st2.rearrange("p (h c) -> p h c", c=HO)[:, :, 0:w],
                        EXP, scale=scale,
                    )
                    if 128 * i >= 512 * jb:
                        # diagonal block sits at local cols [0:128] of both
                        # head-halves: one 3-D-AP tensor_tensor
                        pv2 = pt2.rearrange("p (h c) -> p h c", c=HO)[:, :, 0:P]
                        nc.vector.tensor_tensor(pv2, pv2, maskv, op=MUL)
                    if i < ilast:
                        issue_st(i + 1)
                    for _ in range(pace):
                        if feed:
                            feed.pop(0)()
                    for h in range(2):
                        hh = 2 * t + h
                        o = qoffs[i] - 512 * jb
                        nc.tensor.matmul(
                            otph[h][:, o : o + w],
                            vs[i][:, hh * HC : (hh + 1) * HC],
                            pt2[:, h * HO : h * HO + w],
                            start=(i == 0), stop=(i == ilast),
                        )
                while feed:
                    feed.pop(0)()
                for h in range(2):
                    base = h * HD
                    den = work.tile([1, 512], F32, tag="den", name="den")
                    nc.scalar.copy(den[:], otph[h][HD : HD + 1, :])
                    rec = work.tile([1, 512], F32, tag="rec", name="rec")
                    nc.vector.reciprocal_approx_fast(rec[:], den[:])
                    bc = work.tile([HD, 512], F32, tag="bc", name="bc")
                    nc.gpsimd.partition_broadcast(bc[:], rec[:])
                    nc.vector.tensor_tensor(
                        ot[t][base : base + HD, 512 * jb : 512 * (jb + 1)],
                        otph[h][0:HD, :], bc[:],
                        op=MUL,
                    )

            # steady-state pipeline: Q/K projections of tile t+1 (and, for
            # the last tile, the first output-projection chains) are drained
            # into attention of tile t between the score and PV matmuls
            for th in qk_feed(0, "q"):
                th()
            for th in qk_feed(0, "k"):
                th()
            for t in range(NT):
                if t + 1 < NT:
                    fq, fk = qk_feed(t + 1, "q"), qk_feed(t + 1, "k")
                else:
                    # last pair: jb0 drains the first two out-proj chains up
                    # to i=6; jb1 finishes them (i=7 only needs the jb0-half
                    # of ot[7], normalized during jb1's early slots) and then
                    # runs the st=2 chains, which read only jb0 halves too
                    fq = [
                        (lambda st=st, m=m, i=i: oproj_mm(st, m, i))
                        for (st, m) in ((0, 0), (0, 1))
                        for i in range(NT - 1)
                    ]
                    fk = [
                        (lambda m=m: oproj_mm(0, m, NT - 1)) for m in range(2)
                    ] + [
                        (lambda m=m, i=i: oproj_mm(2, m, i))
                        for m in range(2)
                        for i in range(NT)
                    ]
                attn_half(t, 0, fq, 5)
                attn_half(t, 1, fk, 3)

            for st in range(NT):
                for m in range(2):
                    if st in (0, 2):
                        continue
                    for i in range(NT):
                        oproj_mm(st, m, i)

    nc.compile()
    _NC_CACHE["nc"] = nc
    return nc


def _host_prep(x, wq, wk, wv, wo, freqs_cos, freqs_sin):
    import ml_dtypes

    bf16 = ml_dtypes.bfloat16

    # de-interleave RoPE pairs: permuted col c of head h maps to original
    # column h*64 + (2r if r<32 else 2(r-32)+1)
    r = np.arange(HD)
    src_local = np.where(r < 32, 2 * r, 2 * (r - 32) + 1)
    perm = (np.arange(H)[:, None] * HD + src_local[None, :]).reshape(-1)

    wq_p = np.ascontiguousarray(wq[:, perm]).astype(bf16)
    wk_p = np.ascontiguousarray(wk[:, perm]).astype(bf16)
    wv_c = np.ascontiguousarray(wv).astype(bf16)
    wo_c = np.ascontiguousarray(wo).astype(bf16)

    cos_t = np.ascontiguousarray(freqs_cos.T).astype(np.float32)  # [32, S]
    sin_t = np.ascontiguousarray(freqs_sin.T).astype(np.float32)
    cosf = np.concatenate([cos_t, cos_t, cos_t, cos_t], 0).astype(bf16)  # [128,S]
    sinf = np.concatenate([-sin_t, sin_t, -sin_t, sin_t], 0).astype(bf16)

    kq = np.arange(P)
    mask = ((kq[:, None] // BS) <= (kq[None, :] // BS)).astype(bf16)  # [128,128]
    mask2 = np.concatenate([mask, mask], axis=1)  # [128, 256]

    in_maps = []
    for b in range(NCORES):
        xt = np.ascontiguousarray(x[b].T).astype(bf16)  # [D, S]
        in_maps.append(
            {
                "xt": xt,
                "wq": wq_p,
                "wk": wk_p,
                "wv": wv_c,
                "wo": wo_c,
                "cosf": cosf,
                "sinf": sinf,
                "mask": mask2,
            }
        )
    return in_maps


def kernel(x, wq, wk, wv, wo, freqs_cos, freqs_sin):
    global LAST_RESULT
    x = np.asarray(x, dtype=np.float32)
    wq = np.asarray(wq, dtype=np.float32)
    wk = np.asarray(wk, dtype=np.float32)
    wv = np.asarray(wv, dtype=np.float32)
    wo = np.asarray(wo, dtype=np.float32)
    freqs_cos = np.asarray(freqs_cos, dtype=np.float32)
    freqs_sin = np.asarray(freqs_sin, dtype=np.float32)

    trace = bool(os.environ.get("BASS_TRACE"))
    if trace:
        _install_axon_hooks()
        import concourse.bass_utils as bass_utils

        bass_utils.upload_artifacts = lambda tmpdir: tmpdir  # no-egress sandbox

    from concourse.bass_utils import run_bass_kernel_spmd

    nc = _build_nc()
    in_maps = _host_prep(x, wq, wk, wv, wo, freqs_cos, freqs_sin)
    res = run_bass_kernel_spmd(
        nc, in_maps, core_ids=list(range(NCORES)), trace=trace
    )
    LAST_RESULT = res
    out = np.stack([res.results[b]["out"] for b in range(NCORES)], 0)
    return out.astype(np.float32)


# revision 26
# speedup vs baseline: 1.2230x; 1.0385x over previous
"""Self-contained Trainium2 Bass kernel for batched multi-head attention
with interleaved RoPE and a block-causal mask (block size 8).

Shapes (hardcoded): x [8, 1024, 1024] f32, weights [1024, 1024] f32,
freqs_cos/sin [1024, 32] f32 -> out [8, 1024, 1024] f32.

Sharding: data-parallel over batch, one batch element per NeuronCore (8 cores).

Device algorithm (per core, matmuls in bf16):
  - host pre-transposes x -> XT [D, S] and de-interleaves the RoPE pairing by
    permuting wq/wk columns so each head's 64 dims are [32 real | 32 imag].
  - warmup: a short chain of dummy matmuls keeps the PE HAM clock-gate busy
    while the first input DMAs land, so real matmuls start at 2.4 GHz.
  - QT = Wq^T XT, KT = Wk^T XT  ([D, S] layouts, head-major rows)
  - RoPE in [d, s] layout: rot = t * cosf + swap32(t) * sinf, with the 32-row
    block swap done by SBUF->SBUF DMA and sign folded into the sinf table.
  - V = XT^T Wv in natural [S, D] layout, stored with a ones-column per head
    (V' [S, 65] per head) so the PV matmul also produces the softmax
    denominator as its row 64.
  - attention per head-PAIR (the two heads of a 128-row partition tile) per
    512-wide q-bank jb: for each k-tile i, both heads' transposed score
    tiles ST[k, q] go side by side into ONE PSUM pair-tile [128, 2w]
    (h0 rows of kt use PE row-groups 0-1, h1 rows 2-3), then ONE exp
    (ACT, scale 1/8 folded) covers both heads, ONE tensor_tensor applies
    the block-diagonal mask to both heads via a 3-D access pattern, and
    two PV matmuls accumulate otT[h] = V'^T @ PT per head ([65, 512]).
    The ST matmul for k-tile i+1 is issued before PV of k-tile i so the
    PE streams scores while ACT runs exp (2-deep software pipeline).
  - normalization per head-bank: rec = reciprocal_approx_fast(den row read
    straight from PSUM), partition-broadcast on GPSIMD, fused TT multiply
    PSUM->SBUF.
  - final = outT^T @ Wo streamed back to HBM in f32 (PSUM->SBUF copies on
    the scalar engine to keep DVE free).

PSUM budget (8 banks): 2 projection accumulators + 4 for double-buffered
score pair-tiles + 2 for the per-head otT accumulators.
"""

import os
import sys
import types

import numpy as np

B, S, D, H, HD, BS = 8, 1024, 1024, 16, 64, 8
P = 128
NT = D // P  # 8 partition tiles
NCORES = 8

LAST_RESULT = None  # BassKernelResults of the most recent run (for test harness)


def _install_axon_hooks():
    """Provide antenv.axon_hooks (NTFF profiling hook) when the image lacks it."""
    if "antenv.axon_hooks" in sys.modules:
        return
    try:
        import antenv
        from trn_agent_boot.trn_boot import _ntff_profile_via_ctypes

        mod = types.ModuleType("antenv.axon_hooks")
        hook = _ntff_profile_via_ctypes("/opt/axon/libaxon_pjrt.so")
        mod.get_axon_ntff_profile_hook = lambda: hook
        mod.set_axon_ntff_profile_hook = lambda h: None
        sys.modules["antenv.axon_hooks"] = mod
        antenv.axon_hooks = mod
    except Exception:
        mod = types.ModuleType("antenv.axon_hooks")
        mod.get_axon_ntff_profile_hook = lambda: None
        mod.set_axon_ntff_profile_hook = lambda h: None
        sys.modules["antenv.axon_hooks"] = mod


_NC_CACHE = {}


def _build_nc():
    """Build and compile the Bass graph (one SPMD program for all 8 cores)."""
    if "nc" in _NC_CACHE:
        return _NC_CACHE["nc"]

    import concourse.mybir as mybir
    import concourse.tile as tile
    from concourse import bacc

    BF = mybir.dt.bfloat16
    F32 = mybir.dt.float32
    MUL = mybir.AluOpType.mult
    ADD = mybir.AluOpType.add
    EXP = mybir.ActivationFunctionType.Exp
    COPY = mybir.ActivationFunctionType.Copy

    nc = bacc.Bacc("TRN2", target_bir_lowering=False, debug=False)

    xt_d = nc.dram_tensor("xt", [D, S], BF, kind="ExternalInput")
    wq_d = nc.dram_tensor("wq", [D, D], BF, kind="ExternalInput")
    wk_d = nc.dram_tensor("wk", [D, D], BF, kind="ExternalInput")
    wv_d = nc.dram_tensor("wv", [D, D], BF, kind="ExternalInput")
    wo_d = nc.dram_tensor("wo", [D, D], BF, kind="ExternalInput")
    cos_d = nc.dram_tensor("cosf", [P, S], BF, kind="ExternalInput")
    sin_d = nc.dram_tensor("sinf", [P, S], BF, kind="ExternalInput")
    mask_d = nc.dram_tensor("mask", [P, 2 * P], BF, kind="ExternalInput")
    out_d = nc.dram_tensor("out", [S, D], F32, kind="ExternalOutput")

    HC = HD + 1  # 65: V columns per head incl. the ones column

    with tile.TileContext(nc) as tc:
        with (
            tc.tile_pool(name="big", bufs=1) as big,
            tc.tile_pool(name="prj", bufs=2, space="PSUM") as prj,
            tc.tile_pool(name="stp", bufs=2, space="PSUM") as stp,
            tc.tile_pool(name="otp", bufs=2, space="PSUM") as otp,
            tc.tile_pool(name="work", bufs=2) as work,
            tc.tile_pool(name="ptp", bufs=4) as ptp,
        ):
            xt = [big.tile([P, S], BF, tag=f"xt{j}", name=f"xt{j}") for j in range(NT)]
            wqt = [big.tile([P, D], BF, tag=f"wq{j}", name=f"wq{j}") for j in range(NT)]
            wkt = [big.tile([P, D], BF, tag=f"wk{j}", name=f"wk{j}") for j in range(NT)]
            wvt = [big.tile([P, D], BF, tag=f"wv{j}", name=f"wv{j}") for j in range(NT)]
            wot = [big.tile([P, D], BF, tag=f"wo{j}", name=f"wo{j}") for j in range(NT)]
            qt = [big.tile([P, S], BF, tag=f"qt{t}", name=f"qt{t}") for t in range(NT)]
            kt = [big.tile([P, S], BF, tag=f"kt{t}", name=f"kt{t}") for t in range(NT)]
            vs = [big.tile([P, H * HC], BF, tag=f"vs{t}", name=f"vs{t}") for t in range(NT)]
            ot = [big.tile([P, S], BF, tag=f"ot{t}", name=f"ot{t}") for t in range(NT)]
            cosf = big.tile([P, S], BF, tag="cosf", name="cosf")
            sinf = big.tile([P, S], BF, tag="sinf", name="sinf")
            maskt2 = big.tile([P, 2 * P], BF, tag="mask", name="mask")
            wup = big.tile([P, 512], BF, tag="wup", name="wup")

            # ---- PE warmup: dummy matmul chain with no input deps ----------
            nc.vector.memset(wup[:], 0.0)
            wps = prj.tile([P, 512], F32, tag="prj", name="wps")
            for k in range(30):
                nc.tensor.matmul(
                    wps[:], wup[:, 0:P], wup[:, 0:512],
                    start=(k == 0), stop=(k == 29),
                )
            wsb = work.tile([1, P], F32, tag="wsb", name="wsb")
            nc.vector.tensor_copy(wsb[:], wps[0:1, 0:P])

            # ---- input DMAs in consumption order --------------------------
            for j in range(NT):
                rs = slice(j * P, (j + 1) * P)
                nc.sync.dma_start(xt[j][:], xt_d[rs, :])
                nc.sync.dma_start(wvt[j][:], wv_d[rs, :])
            for j in range(NT):
                rs = slice(j * P, (j + 1) * P)
                nc.sync.dma_start(wqt[j][:], wq_d[rs, :])
                nc.sync.dma_start(wkt[j][:], wk_d[rs, :])
            nc.sync.dma_start(cosf[:], cos_d[:])
            nc.sync.dma_start(sinf[:], sin_d[:])
            nc.sync.dma_start(maskt2[:], mask_d[:])
            for j in range(NT):
                rs = slice(j * P, (j + 1) * P)
                nc.sync.dma_start(wot[j][:], wo_d[rs, :])

            for t in range(NT):
                nc.vector.memset(
                    vs[t].rearrange("p (h c) -> p h c", c=HC)[:, :, HD : HD + 1], 1.0
                )

            # ---- V projection --------------------------------------------
            def v_group(t, m):
                cs = slice(t * P, (t + 1) * P)
                sl = slice(m * 512, (m + 1) * 512)
                pv = prj.tile([P, 512], F32, tag="prj", name="pv")
                for j in range(NT):
                    nc.tensor.matmul(
                        pv[:], xt[j][:, cs], wvt[j][:, sl],
                        start=(j == 0), stop=(j == NT - 1),
                    )
                dst = vs[t].rearrange("p (h c) -> p h c", c=HC)[
                    :, m * 8 : (m + 1) * 8, 0:HD
                ]
                srcv = pv.rearrange("p (h c) -> p h c", c=HD)
                nc.vector.tensor_copy(dst, srcv)

            # first four groups (t=0,1 x m=0,1) j-interleaved so matmuls
            # trickle in densely as the per-j DMAs land (keeps HAM warm);
            # two accumulators borrow idle score-pool banks
            pvs = [
                prj.tile([P, 512], F32, tag="prj", name="pv"),
                prj.tile([P, 512], F32, tag="prj", name="pv"),
                stp.tile([P, 512], F32, tag="st2", name="pv"),
                stp.tile([P, 512], F32, tag="st2", name="pv"),
            ]
            for j in range(NT):
                for g, (t, m) in enumerate(((0, 0), (0, 1), (1, 0), (1, 1))):
                    nc.tensor.matmul(
                        pvs[g][:], xt[j][:, t * P : (t + 1) * P],
                        wvt[j][:, m * 512 : (m + 1) * 512],
                        start=(j == 0), stop=(j == NT - 1),
                    )
            for g, (t, m) in enumerate(((0, 0), (0, 1), (1, 0), (1, 1))):
                dst = vs[t].rearrange("p (h c) -> p h c", c=HC)[
                    :, m * 8 : (m + 1) * 8, 0:HD
                ]
                nc.vector.tensor_copy(dst, pvs[g].rearrange("p (h c) -> p h c", c=HD))
            for t in range(2, NT):
                for m in range(2):
                    v_group(t, m)

            # RoPE helper: per 128-row tile the layout is [h0r, h0i, h1r,
            # h1i] (32 rows each); rot = t*cosf + swap32(t)*sinf (sinf
            # carries the sign)
            def rope(buf_t):
                tr = work.tile([P, S], BF, tag="trot", name="trot")
                for b4 in range(4):
                    sblk = (b4 ^ 1) * 32
                    dblk = b4 * 32
                    nc.sync.dma_start(
                        tr[dblk : dblk + 32, :], buf_t[sblk : sblk + 32, :]
                    )
                nc.vector.tensor_tensor(tr[:], tr[:], sinf[:], op=MUL)
                nc.vector.tensor_tensor(buf_t[:], buf_t[:], cosf[:], op=MUL)
                nc.vector.tensor_tensor(buf_t[:], buf_t[:], tr[:], op=ADD)

            def qk_feed(t, which):
                """Thunks for one Q-or-K projection of tile t: 2 m-groups of
                8 chained matmuls + PSUM->SBUF cast, then RoPE. Drained one
                thunk at a time inside the attention k-loop so the in-order
                PE queue always has independent matmuls before each blocking
                PV matmul."""
                cs = slice(t * P, (t + 1) * P)
                wsrc = wqt if which == "q" else wkt
                dstt = qt[t] if which == "q" else kt[t]
                thunks = []
                cell = {}
                for m in range(2):
                    sl = slice(m * 512, (m + 1) * 512)

                    def mk_mm(j, m=m, sl=sl):
                        def f():
                            if j == 0:
                                cell[m] = prj.tile(
                                    [P, 512], F32, tag="prj", name="pq"
                                )
                            nc.tensor.matmul(
                                cell[m][:], wsrc[j][:, cs], xt[j][:, sl],
                                start=(j == 0), stop=(j == NT - 1),
                            )
                        return f

                    for j in range(NT):
                        thunks.append(mk_mm(j))

                    def mk_cast(m=m, sl=sl):
                        def f():
                            nc.vector.tensor_copy(dstt[:, sl], cell[m][:])
                        return f

                    thunks.append(mk_cast())
                thunks.append(lambda: rope(dstt))
                return thunks

            # ---- output projection pieces (fed partially into the last
            # attention pair): final[s, :] = sum_i ot[i][:, s]^T wo[i]
            oproj_state = {}

            def oproj_mm(st, m, i):
                key = (st, m)
                if key not in oproj_state:
                    oproj_state[key] = prj.tile([P, 512], F32, tag="prj", name="fp")
                fp = oproj_state[key]
                nc.tensor.matmul(
                    fp[:],
                    ot[i][:, st * P : (st + 1) * P],
                    wot[i][:, m * 512 : (m + 1) * 512],
                    start=(i == 0), stop=(i == NT - 1),
                )
                if i == NT - 1:
                    osb = work.tile([P, 512], F32, tag="osb", name="osb")
                    nc.scalar.activation(osb[:], fp[:], COPY)
                    nc.sync.dma_start(
                        out_d[st * P : (st + 1) * P, m * 512 : (m + 1) * 512],
                        osb[:],
                    )

            # ---- attention per head-pair, per 512-wide q-bank jb ----------
            scale = 1.0 / 8.0
            maskv = maskt2.rearrange("p (h c) -> p h c", c=P)

            def attn_half(t, jb, feed, pace):
                ilast = 4 * jb + 3 if jb == 0 else NT - 1
                ks = list(range(ilast + 1))
                widths = [512 - max(0, 128 * i - 512 * jb) for i in ks]
                qoffs = [max(512 * jb, 128 * i) for i in ks]
                otph = [
                    otp.tile([HC, 512], F32, tag="ot", name=f"otp{h}")
                    for h in range(2)
                ]
                st2s = {}
                HO = 512  # h1's half starts at a PSUM bank boundary

                def issue_st(i):
                    w = widths[i]
                    st2 = stp.tile([P, 2 * HO], F32, tag="st2", name="st2")
                    st2s[i] = st2
                    for h in range(2):
                        base = h * HD
                        nc.tensor.matmul(
                            st2[:, h * HO : h * HO + w],
                            kt[t][base : base + HD, 128 * i : 128 * (i + 1)],
                            qt[t][base : base + HD, qoffs[i] : qoffs[i] + w],
                            start=True, stop=True,
                        )

                issue_st(0)
                for i in ks:
                    w = widths[i]
                    st2 = st2s.pop(i)
                    pt2 = ptp.tile([P, 2 * HO], BF, tag="pt2", name="pt2")
                    nc.scalar.activation(
                        pt2.rearrange("p (h c) -> p h c", c=HO)[:, :, 0:w],
                        st2.rearrange("p (h c) -> p h c", c=HO)[:, :, 0:w],
                        EXP, scale=scale,
                    )
                    if 128 * i >= 512 * jb:
                        # diagonal block sits at local cols [0:128] of both
                        # head-halves: one 3-D-AP tensor_tensor
                        pv2 = pt2.rearrange("p (h c) -> p h c", c=HO)[:, :, 0:P]
                        nc.vector.tensor_tensor(pv2, pv2, maskv, op=MUL)
                    if i < ilast:
                        issue_st(i + 1)
                    for _ in range(pace):
                        if feed:
                            feed.pop(0)()
                    for h in range(2):
                        hh = 2 * t + h
                        o = qoffs[i] - 512 * jb
                        nc.tensor.matmul(
                            otph[h][:, o : o + w],
                            vs[i][:, hh * HC : (hh + 1) * HC],
                            pt2[:, h * HO : h * HO + w],
                            start=(i == 0), stop=(i == ilast),
                        )
                while feed:
                    feed.pop(0)()
                for h in range(2):
                    base = h * HD
                    den = work.tile([1, 512], F32, tag="den", name="den")
                    nc.scalar.copy(den[:], otph[h][HD : HD + 1, :])
                    rec = work.tile([1, 512], F32, tag="rec", name="rec")
                    nc.vector.reciprocal_approx_fast(rec[:], den[:])
                    bc = work.tile([HD, 512], F32, tag="bc", name="bc")
                    nc.gpsimd.partition_broadcast(bc[:], rec[:])
                    nc.vector.tensor_tensor(
                        ot[t][base : base + HD, 512 * jb : 512 * (jb + 1)],
                        otph[h][0:HD, :], bc[:],
                        op=MUL,
                    )

            # steady-state pipeline: Q/K projections of tile t+1 (and, for
            # the last tile, the first output-projection chains) are drained
            # into attention of tile t between the score and PV matmuls
            for th in qk_feed(0, "q"):
                th()
            for th in qk_feed(0, "k"):
                th()
            for t in range(NT):
                if t + 1 < NT:
                    fq, fk = qk_feed(t + 1, "q"), qk_feed(t + 1, "k")
                else:
                    # last pair: jb0 drains the first two out-proj chains up
                    # to i=6; jb1 finishes them (i=7 only needs the jb0-half
                    # of ot[7], normalized during jb1's early slots) and then
                    # runs the st=2 chains, which read only jb0 halves too
                    fq = [
                        (lambda st=st, m=m, i=i: oproj_mm(st, m, i))
                        for (st, m) in ((0, 0), (0, 1))
                        for i in range(NT - 1)
                    ]
                    fk = [
                        (lambda m=m: oproj_mm(0, m, NT - 1)) for m in range(2)
                    ] + [
                        (lambda m=m, i=i: oproj_mm(2, m, i))
                        for m in range(2)
                        for i in range(NT)
                    ]
                attn_half(t, 0, fq, 5)
                attn_half(t, 1, fk, 3)

            for st in range(NT):
                for m in range(2):
                    if st in (0, 2):
                        continue
                    for i in range(NT):
                        oproj_mm(st, m, i)

    nc.compile()
    _NC_CACHE["nc"] = nc
    return nc


def _host_prep(x, wq, wk, wv, wo, freqs_cos, freqs_sin):
    import ml_dtypes

    bf16 = ml_dtypes.bfloat16

    # de-interleave RoPE pairs: permuted col c of head h maps to original
    # column h*64 + (2r if r<32 else 2(r-32)+1)
    r = np.arange(HD)
    src_local = np.where(r < 32, 2 * r, 2 * (r - 32) + 1)
    perm = (np.arange(H)[:, None] * HD + src_local[None, :]).reshape(-1)

    wq_p = np.ascontiguousarray(wq[:, perm]).astype(bf16)
    wk_p = np.ascontiguousarray(wk[:, perm]).astype(bf16)
    wv_c = np.ascontiguousarray(wv).astype(bf16)
    wo_c = np.ascontiguousarray(wo).astype(bf16)

    cos_t = np.ascontiguousarray(freqs_cos.T).astype(np.float32)  # [32, S]
    sin_t = np.ascontiguousarray(freqs_sin.T).astype(np.float32)
    cosf = np.concatenate([cos_t, cos_t, cos_t, cos_t], 0).astype(bf16)  # [128,S]
    sinf = np.concatenate([-sin_t, sin_t, -sin_t, sin_t], 0).astype(bf16)

    kq = np.arange(P)
    mask = ((kq[:, None] // BS) <= (kq[None, :] // BS)).astype(bf16)  # [128,128]
    mask2 = np.concatenate([mask, mask], axis=1)  # [128, 256]

    in_maps = []
    for b in range(NCORES):
        xt = np.ascontiguousarray(x[b].T).astype(bf16)  # [D, S]
        in_maps.append(
            {
                "xt": xt,
                "wq": wq_p,
                "wk": wk_p,
                "wv": wv_c,
                "wo": wo_c,
                "cosf": cosf,
                "sinf": sinf,
                "mask": mask2,
            }
        )
    return in_maps


def kernel(x, wq, wk, wv, wo, freqs_cos, freqs_sin):
    global LAST_RESULT
    x = np.asarray(x, dtype=np.float32)
    wq = np.asarray(wq, dtype=np.float32)
    wk = np.asarray(wk, dtype=np.float32)
    wv = np.asarray(wv, dtype=np.float32)
    wo = np.asarray(wo, dtype=np.float32)
    freqs_cos = np.asarray(freqs_cos, dtype=np.float32)
    freqs_sin = np.asarray(freqs_sin, dtype=np.float32)

    trace = bool(os.environ.get("BASS_TRACE"))
    if trace:
        _install_axon_hooks()
        import concourse.bass_utils as bass_utils

        bass_utils.upload_artifacts = lambda tmpdir: tmpdir  # no-egress sandbox

    from concourse.bass_utils import run_bass_kernel_spmd

    nc = _build_nc()
    in_maps = _host_prep(x, wq, wk, wv, wo, freqs_cos, freqs_sin)
    res = run_bass_kernel_spmd(
        nc, in_maps, core_ids=list(range(NCORES)), trace=trace
    )
    LAST_RESULT = res
    out = np.stack([res.results[b]["out"] for b in range(NCORES)], 0)
    return out.astype(np.float32)
